# revision 1
# baseline (speedup 1.0000x reference)
"""Trainium2 Bass kernel for CausalSelfAttention (B=4, T=2048, C=2048, H=16).

Sharding: 8 cores = 4 batches x 2 head-groups (8 heads each).
Each core computes q/k/v projections for its heads, RoPE, causal attention,
and a partial output projection (row-parallel c_proj over its heads' columns).
Host sums the two partials per batch (standard row-parallel TP unshard).

On-chip layout notes:
  - All matmul contractions run with the contracted dim on partitions.
  - Host pre-transposes x and weights so every DMA is contiguous.
  - Scores are computed transposed (s^T[tk, tq]) so softmax normalization
    becomes: partition-sum via ones-matmul + reciprocal + DMA-replicate
    broadcast, and att@v needs no on-chip transposes at all.
  - RoPE rotate-half is a fixed 128x128 signed permutation applied via one
    extra matmul per q/k tile; cos/sin enter as elementwise tables.
"""

import numpy as np
import ml_dtypes

import concourse.bass as bass
import concourse.mybir as mybir
import concourse.tile as tile
from concourse import bacc
from concourse.alu_op_type import AluOpType
from concourse.bass import ds
from concourse.bass_utils import run_bass_kernel_spmd

BF16 = ml_dtypes.bfloat16
F32 = np.float32

B = 4
C = 2048
H = 16
D = 128
HPC = 8          # heads per core
P = 128
CH = 512         # tq chunk width
NCT = C // P     # 16 contraction tiles
AF = mybir.ActivationFunctionType
SCALE = 1.0 / float(np.sqrt(np.float32(D)))


def build_nc(T=2048):
    NCH = T // CH
    dt = mybir.dt
    nc = bacc.Bacc(None, target_bir_lowering=False)

    xT = nc.dram_tensor("xT", [C, T], dt.bfloat16, kind="ExternalInput")
    wq = nc.dram_tensor("wq", [C, HPC * D], dt.bfloat16, kind="ExternalInput")
    wk = nc.dram_tensor("wk", [C, HPC * D], dt.bfloat16, kind="ExternalInput")
    wv = nc.dram_tensor("wv", [C, HPC * D], dt.bfloat16, kind="ExternalInput")
    wp = nc.dram_tensor("wp", [HPC * D, C], dt.bfloat16, kind="ExternalInput")
    ab_a = nc.dram_tensor("ab_a", [D, T], dt.bfloat16, kind="ExternalInput")
    ab_b = nc.dram_tensor("ab_b", [D, T], dt.bfloat16, kind="ExternalInput")
    bq = nc.dram_tensor("bq", [D, HPC], dt.float32, kind="ExternalInput")
    bk = nc.dram_tensor("bk", [D, HPC], dt.float32, kind="ExternalInput")
    bv = nc.dram_tensor("bv", [1, HPC * D], dt.bfloat16, kind="ExternalInput")
    bp = nc.dram_tensor("bp", [1, C], dt.bfloat16, kind="ExternalInput")
    maskm = nc.dram_tensor("maskm", [4, P, CH], dt.bfloat16, kind="ExternalInput")
    pt = nc.dram_tensor("pt", [D, D], dt.bfloat16, kind="ExternalInput")
    onc = nc.dram_tensor("onc", [P, 1], dt.float32, kind="ExternalInput")
    onr = nc.dram_tensor("onr", [1, P], dt.bfloat16, kind="ExternalInput")
    out = nc.dram_tensor("out", [T, C], dt.float32, kind="ExternalOutput")
    scratch = nc.dram_tensor("den_scratch", [NCH, HPC, CH], dt.float32)

    xT_r = xT.rearrange("(ct p) t -> p ct t", p=P)
    wq_r = wq.rearrange("(ct p) d -> p ct d", p=P)
    wk_r = wk.rearrange("(ct p) d -> p ct d", p=P)
    wv_r = wv.rearrange("(ct p) d -> p ct d", p=P)
    wp_r = wp.rearrange("(hc p) o -> p hc o", p=P)

    with tile.TileContext(nc) as tc:
        with (
            tc.tile_pool(name="consts", bufs=1) as consts,
            tc.tile_pool(name="keep", bufs=1) as keep,
        ):
            mask_sb = consts.tile([P, 4, CH], dt.bfloat16)
            pt_sb = consts.tile([D, D], dt.bfloat16)
            bq_sb = consts.tile([D, HPC], dt.float32)
            bk_sb = consts.tile([D, HPC], dt.float32)
            bv_sb = consts.tile([1, HPC * D], dt.bfloat16)
            onc_sb = consts.tile([P, 1], dt.float32)
            onr_sb = consts.tile([1, P], dt.bfloat16)

            def load_consts():
                nc.sync.dma_start(out=mask_sb, in_=maskm.rearrange("m p c -> p m c"))
                nc.sync.dma_start(out=pt_sb, in_=pt[:])
                nc.sync.dma_start(out=bq_sb, in_=bq[:])
                nc.sync.dma_start(out=bk_sb, in_=bk[:])
                nc.sync.dma_start(out=bv_sb, in_=bv[:])
                nc.sync.dma_start(out=onc_sb, in_=onc[:])
                nc.sync.dma_start(out=onr_sb, in_=onr[:])

            yT = keep.tile([P, HPC, T], dt.bfloat16)

            with (
                tc.tile_pool(name="kv", bufs=1) as kvp,
                tc.tile_pool(name="xw", bufs=1) as xwp,
                tc.tile_pool(name="wtp", bufs=2) as wtp,
                tc.tile_pool(name="wvp", bufs=1) as wvp,
                tc.tile_pool(name="work", bufs=4) as work,
                tc.tile_pool(name="qpp", bufs=10) as qpp,
                tc.tile_pool(name="denp", bufs=2) as denp,
                tc.tile_pool(name="ps_acc", bufs=2, space="PSUM") as ps_acc,
                tc.tile_pool(name="ps_rot", bufs=1, space="PSUM") as ps_rot,
                tc.tile_pool(name="ps_s", bufs=3, space="PSUM") as ps_s,
                tc.tile_pool(name="ps_y", bufs=2, space="PSUM") as ps_y,
            ):
                kT = kvp.tile([P, HPC, T], dt.bfloat16)
                vS = kvp.tile([P, HPC, T], dt.bfloat16)

                for j in range(NCH):
                    cols = ds(j * CH, CH)
                    xc = xwp.tile([P, NCT, CH], dt.bfloat16, tag="xc")
                    for cg in range(4):
                        nc.sync.dma_start(
                            out=xc[:, ds(cg * 4, 4), :],
                            in_=xT_r[:, ds(cg * 4, 4), cols],
                        )
                    if j == 0:
                        load_consts()
                    a_sb = work.tile([D, CH], dt.bfloat16, tag="abA", bufs=2)
                    nc.sync.dma_start(out=a_sb, in_=ab_a[:, cols])
                    b_sb = work.tile([D, CH], dt.bfloat16, tag="abB", bufs=2)
                    nc.sync.dma_start(out=b_sb, in_=ab_b[:, cols])

                    qp_tiles = []

                    def emit_rope(raw, dest):
                        # q'/k' = A (.) raw + B (.) (P @ raw), via one PE
                        # matmul for the rotate-half permutation
                        rps = ps_rot.tile([P, CH], dt.float32, tag="rot")
                        nc.tensor.matmul(
                            rps, lhsT=pt_sb, rhs=raw, start=True, stop=True
                        )
                        t1 = work.tile([P, CH], dt.float32, tag="t1", bufs=2)
                        nc.gpsimd.tensor_tensor(
                            out=t1, in0=raw, in1=a_sb, op=AluOpType.mult
                        )
                        t2 = work.tile([P, CH], dt.float32, tag="t2", bufs=2)
                        nc.vector.tensor_tensor(
                            out=t2, in0=rps, in1=b_sb, op=AluOpType.mult
                        )
                        nc.vector.tensor_tensor(
                            out=dest, in0=t1, in1=t2, op=AluOpType.add
                        )

                    pending = None  # one-deep pipeline so rot never stalls PE
                    for qk in range(2):
                        wsrc = wq_r if qk == 0 else wk_r
                        bsrc = bq_sb if qk == 0 else bk_sb
                        for h in range(HPC):
                            wt = wtp.tile([P, NCT, D], dt.bfloat16, tag="wt")
                            nc.sync.dma_start(out=wt, in_=wsrc[:, :, ds(h * D, D)])
                            ps = ps_acc.tile([P, CH], dt.float32, tag="acc")
                            for ct in range(NCT):
                                nc.tensor.matmul(
                                    ps,
                                    lhsT=wt[:, ct, :],
                                    rhs=xc[:, ct, :],
                                    start=(ct == 0),
                                    stop=(ct == NCT - 1),
                                )
                            raw = work.tile([P, CH], dt.bfloat16, tag="raw")
                            nc.vector.tensor_tensor(
                                out=raw,
                                in0=ps,
                                in1=bsrc[:, ds(h, 1)].to_broadcast([P, CH]),
                                op=AluOpType.add,
                            )
                            if qk == 0:
                                dest = qpp.tile([P, CH], dt.bfloat16, tag="qp")
                                qp_tiles.append(dest)
                            else:
                                dest = kT[:, h, cols]
                            if pending is not None:
                                emit_rope(*pending)
                            pending = (raw, dest)
                    emit_rope(*pending)

                    for half in range(2):
                        wvt = wvp.tile([P, NCT, CH], dt.bfloat16, tag="wv")
                        nc.sync.dma_start(out=wvt, in_=wv_r[:, :, ds(half * CH, CH)])
                        for tt in range(4):
                            ps = ps_acc.tile([P, CH], dt.float32, tag="acc")
                            for ct in range(NCT):
                                nc.tensor.matmul(
                                    ps,
                                    lhsT=xc[:, ct, ds(tt * D, D)],
                                    rhs=wvt[:, ct, :],
                                    start=(ct == 0),
                                    stop=False,
                                )
                            nc.tensor.matmul(
                                ps,
                                lhsT=onr_sb,
                                rhs=bv_sb[:, ds(half * CH, CH)],
                                start=False,
                                stop=True,
                            )
                            ti = 4 * j + tt
                            for hh in range(4):
                                h = half * 4 + hh
                                nc.vector.tensor_copy(
                                    out=vS[:, h, ds(ti * D, D)],
                                    in_=ps[:, ds(hh * D, D)],
                                )

                    den_rows = denp.tile([HPC, CH], dt.float32, tag="dr", bufs=1)
                    yraw_tiles = []
                    for h in range(HPC):
                        qp = qp_tiles[h]
                        den_a = denp.tile([P, CH], dt.float32, tag="dena")
                        yps = ps_y.tile([P, CH], dt.float32, tag="y")
                        ntk = 4 * (j + 1)
                        exq = []  # (ex, i, off) pending y-matmuls
                        for i in range(ntk):
                            sps = ps_s.tile([P, CH], dt.float32, tag="s")
                            m = i - 4 * j
                            off = max(m, 0) * D  # valid tq cols start here
                            w = CH - off
                            nc.tensor.matmul(
                                sps[:, ds(off, w)],
                                lhsT=kT[:, h, ds(i * D, D)],
                                rhs=qp[:, ds(off, w)],
                                start=True,
                                stop=True,
                            )
                            ex = work.tile([P, CH], dt.bfloat16, tag="ex", bufs=6)
                            nc.scalar.activation(
                                ex[:, ds(off, w)], sps[:, ds(off, w)],
                                AF.Exp, scale=SCALE,
                            )
                            if m >= 0:
                                # triangular mask on the diagonal 128-block
                                nc.vector.tensor_tensor(
                                    out=ex[:, ds(off, D)],
                                    in0=ex[:, ds(off, D)],
                                    in1=mask_sb[:, 0, ds(0, D)],
                                    op=AluOpType.mult,
                                )
                            if i == 0:
                                nc.vector.tensor_copy(
                                    out=den_a[:, ds(off, w)], in_=ex[:, ds(off, w)]
                                )
                                if off > 0:
                                    nc.vector.memset(den_a[:, ds(0, off)], 0.0)
                            else:
                                nc.vector.tensor_tensor(
                                    out=den_a[:, ds(off, w)],
                                    in0=den_a[:, ds(off, w)],
                                    in1=ex[:, ds(off, w)],
                                    op=AluOpType.add,
                                )
                            exq.append((ex, i, off))
                            if len(exq) > 2:
                                pex, pi, poff = exq.pop(0)
                                nc.tensor.matmul(
                                    yps[:, ds(poff, CH - poff)],
                                    lhsT=vS[:, h, ds(pi * D, D)],
                                    rhs=pex[:, ds(poff, CH - poff)],
                                    start=(pi == 0),
                                    stop=False,
                                )
                        while exq:
                            pex, pi, poff = exq.pop(0)
                            nc.tensor.matmul(
                                yps[:, ds(poff, CH - poff)],
                                lhsT=vS[:, h, ds(pi * D, D)],
                                rhs=pex[:, ds(poff, CH - poff)],
                                start=(pi == 0),
                                stop=(not exq),
                            )
                        yraw = work.tile([P, CH], dt.bfloat16, tag="yraw", bufs=10)
                        nc.vector.tensor_copy(out=yraw, in_=yps)
                        yraw_tiles.append(yraw)
                        dsum = ps_y.tile([1, CH], dt.float32, tag="y")
                        nc.tensor.matmul(
                            dsum, lhsT=onc_sb, rhs=den_a, start=True, stop=True
                        )
                        dstage = denp.tile([1, CH], dt.float32, tag="dstage")
                        nc.scalar.activation(dstage, dsum, AF.Copy)
                        nc.sync.dma_start(out=den_rows[ds(h, 1), :], in_=dstage)
                    # one wide reciprocal for all 8 heads, then per-head
                    # partition-broadcast via DMA replicate
                    rec8 = denp.tile([HPC, CH], dt.float32, tag="rec8", bufs=1)
                    nc.vector.reciprocal(rec8, den_rows)
                    nc.sync.dma_start(out=scratch[j], in_=rec8)
                    for h in range(HPC):
                        rbc = work.tile([P, CH], dt.float32, tag="rbc", bufs=2)
                        nc.sync.dma_start(
                            out=rbc,
                            in_=scratch[j, h][None, :].to_broadcast([P, CH]),
                        )
                        nc.gpsimd.tensor_tensor(
                            out=yT[:, h, cols],
                            in0=yraw_tiles[h],
                            in1=rbc,
                            op=AluOpType.mult,
                        )

            with (
                tc.tile_pool(name="wpp", bufs=1) as wpp,
                tc.tile_pool(name="outp", bufs=3) as outp,
                tc.tile_pool(name="ps_o", bufs=4, space="PSUM") as ps_o,
            ):
                wps = wpp.tile([P, HPC, C], dt.bfloat16)
                for hc in range(HPC):
                    nc.sync.dma_start(
                        out=wps[:, ds(hc, 1), :], in_=wp_r[:, ds(hc, 1), :]
                    )
                bp_sb = wpp.tile([1, C], dt.bfloat16)
                nc.sync.dma_start(out=bp_sb, in_=bp[:])
                for tt in range(T // P):
                    for oc in range(C // CH):
                        ps = ps_o.tile([P, CH], dt.float32, tag="o")
                        for hc in range(HPC):
                            nc.tensor.matmul(
                                ps,
                                lhsT=yT[:, hc, ds(tt * D, D)],
                                rhs=wps[:, hc, ds(oc * CH, CH)],
                                start=(hc == 0),
                                stop=False,
                            )
                        nc.tensor.matmul(
                            ps,
                            lhsT=onr_sb,
                            rhs=bp_sb[:, ds(oc * CH, CH)],
                            start=False,
                            stop=True,
                        )
                        ot = outp.tile([P, CH], dt.float32, tag="ot")
                        nc.vector.tensor_copy(out=ot, in_=ps)
                        nc.sync.dma_start(
                            out=out[ds(tt * P, P), ds(oc * CH, CH)], in_=ot
                        )
    nc.compile()
    return nc


def _rope_tables(T):
    inv_freq = (
        1.0 / (10000.0 ** (np.arange(0, D, 2, dtype=np.float32) / np.float32(D)))
    ).astype(np.float32)
    t = np.arange(T, dtype=np.float32)
    freqs = t[:, None] * inv_freq[None, :]
    emb = np.concatenate((freqs, freqs), axis=-1)
    cos = np.cos(emb).astype(np.float32)
    sin = np.sin(emb).astype(np.float32)
    A = np.ascontiguousarray((cos + sin).T).astype(BF16)
    Bt = np.ascontiguousarray((cos - sin).T).astype(BF16)
    return A, Bt


def _rot_pt():
    Pm = np.zeros((D, D), dtype=np.float32)
    for d in range(64):
        Pm[d, 2 * d + 1] = -1.0
        Pm[64 + d, 2 * d] = 1.0
    return np.ascontiguousarray(Pm.T).astype(BF16)


def _maskm():
    # maskm[m, p, c] = -1e30 where tq < tk for a diagonal-region tile at
    # relative position m (tk tile i = 4j+m within tq chunk j): c < 128m + p
    row = np.arange(P)[:, None]
    col = np.arange(CH)[None, :]
    out = np.zeros((4, P, CH), dtype=np.float32)
    for m in range(4):
        out[m] = np.where(col < m * P + row, 0.0, 1.0)
    return out.astype(BF16)


def make_in_maps(x, w_attn, b_attn, w_proj, b_proj, T=2048):
    A, Bt = _rope_tables(T)
    pt = _rot_pt()
    maskm = _maskm()
    onc = np.ones((P, 1), dtype=np.float32)
    onr = np.ones((1, P), dtype=BF16)
    in_maps = []
    for core in range(8):
        b, g = core // 2, core % 2
        gs = slice(g * 1024, (g + 1) * 1024)
        bp_eff = b_proj if g == 0 else np.zeros_like(b_proj)
        in_maps.append(
            {
                "xT": np.ascontiguousarray(x[b][:T].T).astype(BF16),
                "wq": np.ascontiguousarray(w_attn[gs, :].T).astype(BF16),
                "wk": np.ascontiguousarray(w_attn[2048:4096][gs, :].T).astype(BF16),
                "wv": np.ascontiguousarray(w_attn[4096:6144][gs, :].T).astype(BF16),
                "wp": np.ascontiguousarray(w_proj[:, gs].T).astype(BF16),
                "ab_a": A,
                "ab_b": Bt,
                "bq": np.ascontiguousarray(
                    b_attn[gs].reshape(HPC, D).T
                ).astype(np.float32),
                "bk": np.ascontiguousarray(
                    b_attn[2048:4096][gs].reshape(HPC, D).T
                ).astype(np.float32),
                "bv": b_attn[4096:6144][gs].reshape(1, HPC * D).astype(BF16),
                "bp": bp_eff.reshape(1, C).astype(BF16),
                "maskm": maskm,
                "pt": pt,
                "onc": onc,
                "onr": onr,
            }
        )
    return in_maps


_NC_CACHE = {}


def run(x, w_attn, b_attn, w_proj, b_proj, trace=False, trace_cores=None):
    T = x.shape[1]
    if T not in _NC_CACHE:
        _NC_CACHE[T] = build_nc(T)
    nc = _NC_CACHE[T]
    in_maps = make_in_maps(
        np.asarray(x, dtype=np.float32),
        np.asarray(w_attn, dtype=np.float32),
        np.asarray(b_attn, dtype=np.float32),
        np.asarray(w_proj, dtype=np.float32),
        np.asarray(b_proj, dtype=np.float32),
        T=T,
    )
    res = run_bass_kernel_spmd(
        nc, in_maps, core_ids=list(range(8)), trace=trace, trace_cores=trace_cores
    )
    T_, C_ = in_maps[0]["xT"].shape[1], C
    out = np.zeros((B, T_, C_), dtype=np.float32)
    for b in range(B):
        out[b] = res.results[2 * b]["out"] + res.results[2 * b + 1]["out"]
    return out, res


def kernel(x, w_attn, b_attn, w_proj, b_proj):
    out, _ = run(x, w_attn, b_attn, w_proj, b_proj, trace=False)
    return out

